# revision 38
# baseline (speedup 1.0000x reference)
"""Multi-head attention (B=2, S=4096, HIDDEN=512, HEADS=8) on 8 TRN2 NeuronCores.

Sharding: 8 cores = 2 batches x 4 head-groups (2 heads each).
Core c handles batch b = c//4 and heads {2g, 2g+1} where g = c%4
(projection feature slice [g*128, (g+1)*128)).

Per-core kernel, all-bf16 matmul datapath (host pre-converts x and the
weights to bf16; PSUM accumulation stays fp32):
  - K^T/Q^T projections from x^T chunks (lhsT = W chunks, N=512)
  - V projected directly into natural [t, d] layout (lhsT = x^T chunk
    slices), with zero-padded W columns + a broadcast bias row so the
    per-head ones column (softmax denominator) appears for free
  - scores S^T[t, s] = K^T-block^T Q^T chunk in PSUM (fp32, exact)
  - softmax exp split across two engines:
      ACT:  exact exp activation (fp32 -> bf16)
      DVE:  one-pass Schraudolph exp2: bf16_bitcast(int16(
        x*(16*log2e) + B)) -- the f32->int16 convert rounds to nearest,
        the int16 bit pattern IS the bf16 exponential approximation
  - PV in swapped orientation: ctx[s, d+1] accumulates pt-block^T @ vp,
    N=65 per matmul (full 128-wide stationary dim) -- half the PE rows
    of the [d, s] orientation.  PSUM accumulation groups are 2KB-bank
    granular in the sim (start=True wipes the whole bank row), so each
    (query-block, head) group runs alone on its bank, all groups of a
    bank at the same columns so the hazard tracker serializes them.
  - normalize: l sits in ctx column 64; per-partition reciprocal +
    tensor_scalar multiply (queries live on partitions here)
  - ctx transposed back to [d, s] by the DMA crossbar (2-byte
    transpose, no PE/DVE cost)
  - output projection per 128-query block -> partial [S, 512]
Host sums the 4 partials per batch and adds bo.
"""

import sys

import numpy as np

B, S, HID, HEADS, HD = 2, 4096, 512, 8, 64
FSL = 128          # features per core = 2 heads * 64
NCORES = 8
QC = 512           # query-chunk width
NTB = S // 128     # 32 key blocks
NQC = S // QC      # 8 query chunks
Q1_EARLY = 0       # qc=1 tb2-pairs whose QK/exp ride the projection phase

LOG2E = 1.4426950408889634
EXP_S = 16.0 * LOG2E          # 128*log2e * scale(1/8)
EXP_B = 16256.0 - 7.5         # zero-mean-tuned Schraudolph constant

# exp engine assignment patterns, indexed by a global tile counter.
# A = ACT exact exp, D = DVE schraudolph (GPSIMD cannot read PSUM).
# Phase A keeps DVE free for projection moves; phase B rebalances.
EXP_PATTERN_A = ("A", "D")
EXP_PATTERN_B = ("A", "D", "A", "D", "A", "D", "A", "A", "D")

_PROGRAM = None


def _ensure_imports():
    try:
        import concourse  # noqa: F401
    except ImportError:
        sys.path.insert(0, "/opt/trn_rl_repo")


def _build_program():
    _ensure_imports()
    import concourse.bacc as bacc
    import concourse.mybir as mybir
    import concourse.tile as tile

    f32 = mybir.dt.float32
    bf16 = mybir.dt.bfloat16
    fp8 = mybir.dt.float8e4
    i16 = mybir.dt.int16
    PM = mybir.MatmulPerfMode
    AF = mybir.ActivationFunctionType
    ALU = mybir.AluOpType

    nc = bacc.Bacc(
        "TRN2",
        target_bir_lowering=False,
        debug=False,
        enable_asserts=False,
        num_devices=NCORES,
    )

    xT = nc.dram_tensor("xT", [HID, S], bf16, kind="ExternalInput").ap()
    wqT = nc.dram_tensor("wqT", [HID, FSL], bf16, kind="ExternalInput").ap()
    wkT = nc.dram_tensor("wkT", [HID, FSL], bf16, kind="ExternalInput").ap()
    wvT = nc.dram_tensor("wvT", [HID, 130], bf16, kind="ExternalInput").ap()
    woT = nc.dram_tensor("woT", [FSL, HID], bf16, kind="ExternalInput").ap()
    bq = nc.dram_tensor("bq", [FSL, 1], f32, kind="ExternalInput").ap()
    bk = nc.dram_tensor("bk", [FSL, 1], f32, kind="ExternalInput").ap()
    bvr = nc.dram_tensor("bvr", [1, 130], bf16, kind="ExternalInput").ap()
    out = nc.dram_tensor("out", [S, HID], bf16, kind="ExternalOutput").ap()

    exp_idx = [0]
    exp_phase = ["A"]
    pt_map = {}

    with tile.TileContext(nc) as tc:
        with (
            tc.tile_pool(name="persist", bufs=1) as pp,
            tc.tile_pool(name="vp_pool", bufs=NTB) as vpp,
            tc.tile_pool(name="pspersist", bufs=1, space="PSUM") as psp,
            tc.tile_pool(name="stp", bufs=2, space="PSUM") as stp,
            tc.tile_pool(name="pt_pool", bufs=44) as ptp,
            tc.tile_pool(name="w_pool", bufs=1) as wp,
            tc.tile_pool(name="small", bufs=2) as sp,
            tc.tile_pool(name="out_pool", bufs=2) as obp,
        ):
            # fp8 K/Q: flat [d, t] staging + DoubleRow layout [32, h, j, t]
            ktf = pp.tile([FSL, S], fp8, tag="ktf")
            qtf = pp.tile([FSL, S], fp8, tag="qtf")
            kt8 = pp.tile([32, 2, 2, S], fp8, tag="kt8")
            qt8 = pp.tile([32, 2, 2, S], fp8, tag="qt8")
            wo_sb = pp.tile([FSL, HID], bf16, tag="wo_sb")
            bq_sb = pp.tile([FSL, 1], f32, tag="bq_sb")
            bk_sb = pp.tile([FSL, 1], f32, tag="bk_sb")
            bvr_sb = pp.tile([1, 130], bf16, tag="bvr_sb")
            ones_row = pp.tile([1, 128], bf16, tag="ones_row")

            # persistent PSUM banks; one live accumulation group per bank
            # (sim group state is bank-granular):
            #  ctxA/ctxB: PV slot groups, always at cols 0:65
            #  mix: K/Q projection psum (phase A), po output proj (tails)
            #  vpb: V projection psum (phase A)
            ctxA = psp.tile([128, 512], f32, tag="ctxA")
            mix = psp.tile([128, 512], f32, tag="mix")


            # small DMAs ride the SWDGE ring
            nc.gpsimd.dma_start(bq_sb[:], bq[:])
            nc.gpsimd.dma_start(bk_sb[:], bk[:])
            nc.gpsimd.dma_start(bvr_sb[:], bvr[:])
            nc.gpsimd.memset(ones_row[:], 1.0)

            # weights first on the sync ring: tiny, needed before projections
            wk_t = wp.tile([128, 4, FSL], bf16, tag="wk_t")
            nc.sync.dma_start(wk_t[:], wkT.rearrange("(i p) f -> p i f", p=128))
            wq_t = wp.tile([128, 4, FSL], bf16, tag="wq_t")
            nc.sync.dma_start(wq_t[:], wqT.rearrange("(i p) f -> p i f", p=128))
            wv_t = wp.tile([128, 4, 130], bf16, tag="wv_t")
            nc.sync.dma_start(wv_t[:], wvT.rearrange("(i p) f -> p i f", p=128))
            nc.sync.dma_start(wo_sb[:], woT[:])

            vp_tiles = []

            def emit_qk_exp(qc, h, tbs):
                # one score tile covering up to 3 t-blocks; [128, 3, 512]
                # tiles (3 banks, bufs=2) amortize the exp instruction
                # overhead over 1536 lanes while keeping pipeline depth 2
                st = stp.tile([128, 2, QC], f32, tag="stA", bufs=3,
                              name="st")
                pt = ptp.tile([128, 2, QC], bf16, tag="ptA", bufs=66,
                              name="pt")
                n = len(tbs)
                for j, tb in enumerate(tbs):
                    nc.tensor.matmul(
                        st[:, j, :],
                        kt8[:, h, :, tb * 128:(tb + 1) * 128],
                        qt8[:, h, :, qc * QC:(qc + 1) * QC],
                        start=True, stop=True, perf_mode=PM.DoubleRow)
                pat = EXP_PATTERN_A if exp_phase[0] == "A" else EXP_PATTERN_B
                eng = pat[exp_idx[0] % len(pat)]
                exp_idx[0] += 1
                if eng == "A":
                    nc.scalar.activation(pt[:, 0:n, :], st[:, 0:n, :], AF.Exp,
                                         scale=float(HD) ** -0.5)
                else:
                    nc.vector.tensor_scalar(pt[:, 0:n, :].bitcast(i16),
                                            st[:, 0:n, :],
                                            EXP_S, EXP_B, ALU.mult, ALU.add)
                for j, tb in enumerate(tbs):
                    pt_map[(qc, h, tb)] = pt[:, j, :]

            def emit_pv_tail(qc, filler=()):
                # filler: (qc', h, tbs) QK/exp emissions interleaved between
                # PV groups to keep all engines fed
                filler = list(filler)
                nfill = len(filler)
                fi = 0
                ctxn = [sp.tile([128, 2, HD], bf16, tag=f"cn{sb}",
                                name=f"cn{sb}") for sb in range(4)]
                ctxTs = sp.tile([128, 512], bf16, tag="ctxTs", name="ctxTs")
                for g in range(8):
                    sb, h = g // 2, g % 2
                    slot = ctxA[:, 0:65]
                    for tb in range(NTB):
                        pt = pt_map[(qc, h, tb)]
                        nc.tensor.matmul(
                            slot,
                            pt[:, sb * 128:(sb + 1) * 128],
                            vp_tiles[tb][:, h * 65:(h + 1) * 65],
                            start=(tb == 0), stop=(tb == NTB - 1))
                    # stage the slot out fast so the next group can start;
                    # normalize off-bank from the SBUF copy
                    cg = sp.tile([128, 65], f32, tag="cg", bufs=3, name="cg")
                    if g % 2 == 0:
                        nc.scalar.copy(cg[:], slot)
                    else:
                        nc.vector.tensor_copy(cg[:], slot)
                    r1 = sp.tile([128, 1], f32, tag="r1", bufs=3, name="r1")
                    nc.vector.reciprocal(r1[:], cg[:, 64:65])
                    nc.vector.tensor_scalar(
                        ctxn[sb][:, h, :], cg[:, 0:HD], r1[:],
                        None, ALU.mult)
                    if h == 1:
                        # [128s, (2h,64d)] -> [(2h,64d), 128s] on the DMA xbar
                        nc.sync.dma_start_transpose(
                            ctxTs[:, sb * 128:(sb + 1) * 128], ctxn[sb][:])
                        col = qc * QC + sb * 128
                        nc.tensor.matmul(
                            mix[:, :], ctxTs[:, sb * 128:(sb + 1) * 128],
                            wo_sb[:], start=True, stop=True)
                        ob = obp.tile([128, HID], bf16, tag="ob", name="ob")
                        if sb % 2 == 0:
                            nc.scalar.copy(ob[:], mix[:, :])
                        else:
                            nc.vector.tensor_copy(ob[:], mix[:, :])
                        nc.sync.dma_start(out[col:col + 128, :], ob[:])
                    want = nfill * (g + 1) // 8
                    while fi < want:
                        emit_qk_exp(*filler[fi])
                        fi += 1

            # ---------------- phase A: projections + qc0 QK/exp ------------
            qc0_cursor = [0]

            def qc0_chunks_upto(tb_max):
                outl = []
                while True:
                    tb = qc0_cursor[0]
                    n = min(2, NTB - tb, tb_max - tb)
                    if n <= 0 or (n < 2 and tb + n < NTB):
                        break
                    outl.append(tuple(range(tb, tb + n)))
                    qc0_cursor[0] += n
                return outl

            for t8 in range(NQC):
                cs = slice(t8 * QC, (t8 + 1) * QC)
                xc = wp.tile([128, 4, QC], bf16, tag="xc", bufs=3)
                nc.sync.dma_start(
                    xc[:], xT[:, cs].rearrange("(i p) t -> p i t", p=128))
                xcs = [xc[:, i, :] for i in range(4)]
                for w_t, bias_sb, dst, dst8, pbank in (
                        (wk_t, bk_sb, ktf, kt8, ctxA), (wq_t, bq_sb, qtf, qt8, mix)):
                    pb = pbank[:, :]
                    for i in range(4):
                        nc.tensor.matmul(
                            pb, w_t[:, i, :], xcs[i],
                            start=(i == 0), stop=(i == 3))
                    nc.scalar.add(dst[:, cs], pb, bias_sb[:])
                    for h in range(2):
                        for j in range(2):
                            eng = nc.gpsimd if j == 0 else nc.sync
                            eng.dma_start(
                                dst8[:, h, j, cs],
                                dst[h * 64 + j * 32:h * 64 + j * 32 + 32, cs])
                for tl in range(4):
                    tb = t8 * 4 + tl
                    vps = stp.tile([128, 2, QC], f32, tag="stA", bufs=3,
                                   name="vps")[:, 0, 0:130]
                    for i in range(4):
                        nc.tensor.matmul(
                            vps, xcs[i][:, tl * 128:(tl + 1) * 128], wv_t[:, i, :],
                            start=(i == 0), stop=False)
                    nc.tensor.matmul(vps, ones_row[:], bvr_sb[:],
                                     start=False, stop=True)
                    vp = vpp.tile([128, 130], bf16, tag="vp")
                    if tb % 2 == 0:
                        nc.scalar.copy(vp[:], vps)
                    else:
                        nc.vector.tensor_copy(vp[:], vps)
                    vp_tiles.append(vp)
                # qc0 scores/exp for every complete triple now available
                for tbs in qc0_chunks_upto(4 * t8 + 4):
                    for h in range(2):
                        emit_qk_exp(0, h, tbs)
            exp_phase[0] = "B"

            # ---------------- phase B: attention + output projection -------
            def chunked(tb0, off=0):
                outl, tb = [], tb0
                while tb < NTB:
                    n = min(2, NTB - tb)
                    outl.append(tuple(range(tb, tb + n)))
                    tb += n
                return outl

            for qc in range(NQC):
                filler = []
                if qc + 1 < NQC:
                    ts = 2 * Q1_EARLY if qc + 1 == 1 else 0
                    filler = [(qc + 1, h, tbs)
                              for tbs in chunked(ts) for h in range(2)]
                emit_pv_tail(qc, filler)

    nc.compile()
    return nc


def _get_program():
    global _PROGRAM
    if _PROGRAM is None:
        _PROGRAM = _build_program()
    return _PROGRAM


def kernel(**inputs):
    _ensure_imports()
    import ml_dtypes
    from concourse import bass_utils

    bf = ml_dtypes.bfloat16
    x = np.ascontiguousarray(np.asarray(inputs["x"], dtype=np.float32))
    Wq = np.asarray(inputs["Wq"], dtype=np.float32)
    Wk = np.asarray(inputs["Wk"], dtype=np.float32)
    Wv = np.asarray(inputs["Wv"], dtype=np.float32)
    Wo = np.asarray(inputs["Wo"], dtype=np.float32)
    bq = np.asarray(inputs["bq"], dtype=np.float32)
    bk = np.asarray(inputs["bk"], dtype=np.float32)
    bv = np.asarray(inputs["bv"], dtype=np.float32)
    bo = np.asarray(inputs["bo"], dtype=np.float32)

    nc = _get_program()

    wqT_full = np.ascontiguousarray(Wq.T)
    wkT_full = np.ascontiguousarray(Wk.T)
    wvT_full = np.ascontiguousarray(Wv.T)
    woT_full = np.ascontiguousarray(Wo.T)

    in_maps = []
    for c in range(NCORES):
        b, g = divmod(c, 4)
        fs = slice(g * FSL, (g + 1) * FSL)
        wv_aug = np.zeros((HID, 130), np.float32)
        wv_aug[:, 0:64] = wvT_full[:, g * FSL:g * FSL + 64]
        wv_aug[:, 65:129] = wvT_full[:, g * FSL + 64:(g + 1) * FSL]
        bv_aug = np.zeros((1, 130), np.float32)
        bv_aug[0, 0:64] = bv[g * FSL:g * FSL + 64]
        bv_aug[0, 64] = 1.0
        bv_aug[0, 65:129] = bv[g * FSL + 64:(g + 1) * FSL]
        bv_aug[0, 129] = 1.0
        in_maps.append({
            "xT": np.ascontiguousarray(x[b].T.astype(bf)),
            "wqT": np.ascontiguousarray(wqT_full[:, fs].astype(bf)),
            "wkT": np.ascontiguousarray(wkT_full[:, fs].astype(bf)),
            "wvT": np.ascontiguousarray(wv_aug.astype(bf)),
            "woT": np.ascontiguousarray(woT_full[fs, :].astype(bf)),
            "bq": np.ascontiguousarray(bq[fs].reshape(FSL, 1)),
            "bk": np.ascontiguousarray(bk[fs].reshape(FSL, 1)),
            "bvr": bv_aug.astype(bf),
        })

    res = bass_utils.run_bass_kernel_spmd(nc, in_maps,
                                          core_ids=list(range(NCORES)))
    outs = [np.asarray(r["out"], dtype=np.float32) for r in res.results]

    full = np.empty((B, S, HID), dtype=np.float32)
    for b in range(B):
        full[b] = outs[4 * b] + outs[4 * b + 1] + outs[4 * b + 2] + outs[4 * b + 3]
        full[b] += bo
    return full


# revision 39
# speedup vs baseline: 1.0149x; 1.0149x over previous
"""Multi-head attention (B=2, S=4096, HIDDEN=512, HEADS=8) on 8 TRN2 NeuronCores.

Sharding: 8 cores = 2 batches x 4 head-groups (2 heads each).
Core c handles batch b = c//4 and heads {2g, 2g+1} where g = c%4
(projection feature slice [g*128, (g+1)*128)).

Per-core kernel, all-bf16 matmul datapath (host pre-converts x and the
weights to bf16; PSUM accumulation stays fp32):
  - K^T/Q^T projections from x^T chunks (lhsT = W chunks, N=512)
  - V projected directly into natural [t, d] layout (lhsT = x^T chunk
    slices), with zero-padded W columns + a broadcast bias row so the
    per-head ones column (softmax denominator) appears for free
  - scores S^T[t, s] = K^T-block^T Q^T chunk in PSUM (fp32, exact)
  - softmax exp split across two engines:
      ACT:  exact exp activation (fp32 -> bf16)
      DVE:  one-pass Schraudolph exp2: bf16_bitcast(int16(
        x*(16*log2e) + B)) -- the f32->int16 convert rounds to nearest,
        the int16 bit pattern IS the bf16 exponential approximation
  - PV in swapped orientation: ctx[s, d+1] accumulates pt-block^T @ vp,
    N=65 per matmul (full 128-wide stationary dim) -- half the PE rows
    of the [d, s] orientation.  PSUM accumulation groups are 2KB-bank
    granular in the sim (start=True wipes the whole bank row), so each
    (query-block, head) group runs alone on its bank, all groups of a
    bank at the same columns so the hazard tracker serializes them.
  - normalize: l sits in ctx column 64; per-partition reciprocal +
    tensor_scalar multiply (queries live on partitions here)
  - ctx transposed back to [d, s] by the DMA crossbar (2-byte
    transpose, no PE/DVE cost)
  - output projection per 128-query block -> partial [S, 512]
Host sums the 4 partials per batch and adds bo.
"""

import sys

import numpy as np

B, S, HID, HEADS, HD = 2, 4096, 512, 8, 64
FSL = 128          # features per core = 2 heads * 64
NCORES = 8
QC = 512           # query-chunk width
NTB = S // 128     # 32 key blocks
NQC = S // QC      # 8 query chunks
Q1_EARLY = 0       # qc=1 tb2-pairs whose QK/exp ride the projection phase

LOG2E = 1.4426950408889634
EXP_S = 16.0 * LOG2E          # 128*log2e * scale(1/8)
EXP_B = 16256.0 - 7.5         # zero-mean-tuned Schraudolph constant

# exp engine assignment patterns, indexed by a global tile counter.
# A = ACT exact exp, D = DVE schraudolph (GPSIMD cannot read PSUM).
# Phase A keeps DVE free for projection moves; phase B rebalances.
EXP_PATTERN_A = ("A", "D")
EXP_PATTERN_B = ("A", "D")

_PROGRAM = None


def _ensure_imports():
    try:
        import concourse  # noqa: F401
    except ImportError:
        sys.path.insert(0, "/opt/trn_rl_repo")


def _build_program():
    _ensure_imports()
    import concourse.bacc as bacc
    import concourse.mybir as mybir
    import concourse.tile as tile

    f32 = mybir.dt.float32
    bf16 = mybir.dt.bfloat16
    fp8 = mybir.dt.float8e4
    i16 = mybir.dt.int16
    PM = mybir.MatmulPerfMode
    AF = mybir.ActivationFunctionType
    ALU = mybir.AluOpType

    nc = bacc.Bacc(
        "TRN2",
        target_bir_lowering=False,
        debug=False,
        enable_asserts=False,
        num_devices=NCORES,
    )

    xT = nc.dram_tensor("xT", [HID, S], bf16, kind="ExternalInput").ap()
    wqT = nc.dram_tensor("wqT", [HID, FSL], bf16, kind="ExternalInput").ap()
    wkT = nc.dram_tensor("wkT", [HID, FSL], bf16, kind="ExternalInput").ap()
    wvT = nc.dram_tensor("wvT", [HID, 130], bf16, kind="ExternalInput").ap()
    woT = nc.dram_tensor("woT", [FSL, HID], bf16, kind="ExternalInput").ap()
    bq = nc.dram_tensor("bq", [FSL, 1], f32, kind="ExternalInput").ap()
    bk = nc.dram_tensor("bk", [FSL, 1], f32, kind="ExternalInput").ap()
    bvr = nc.dram_tensor("bvr", [1, 130], bf16, kind="ExternalInput").ap()
    out = nc.dram_tensor("out", [S, HID], bf16, kind="ExternalOutput").ap()

    exp_idx = [0]
    exp_phase = ["A"]
    pt_map = {}

    with tile.TileContext(nc) as tc:
        with (
            tc.tile_pool(name="persist", bufs=1) as pp,
            tc.tile_pool(name="vp_pool", bufs=NTB) as vpp,
            tc.tile_pool(name="pspersist", bufs=1, space="PSUM") as psp,
            tc.tile_pool(name="stp", bufs=2, space="PSUM") as stp,
            tc.tile_pool(name="pt_pool", bufs=44) as ptp,
            tc.tile_pool(name="w_pool", bufs=1) as wp,
            tc.tile_pool(name="small", bufs=2) as sp,
            tc.tile_pool(name="out_pool", bufs=2) as obp,
        ):
            # fp8 K/Q: flat [d, t] staging + DoubleRow layout [32, h, j, t]
            ktf = pp.tile([FSL, S], fp8, tag="ktf")
            qtf = pp.tile([FSL, S], fp8, tag="qtf")
            kt8 = pp.tile([32, 2, 2, S], fp8, tag="kt8")
            qt8 = pp.tile([32, 2, 2, S], fp8, tag="qt8")
            wo_sb = pp.tile([FSL, HID], bf16, tag="wo_sb")
            bq_sb = pp.tile([FSL, 1], f32, tag="bq_sb")
            bk_sb = pp.tile([FSL, 1], f32, tag="bk_sb")
            bvr_sb = pp.tile([1, 130], bf16, tag="bvr_sb")
            ones_row = pp.tile([1, 128], bf16, tag="ones_row")

            # persistent PSUM banks; one live accumulation group per bank
            # (sim group state is bank-granular):
            #  ctxA/ctxB: PV slot groups, always at cols 0:65
            #  mix: K/Q projection psum (phase A), po output proj (tails)
            #  vpb: V projection psum (phase A)
            ctxA = psp.tile([128, 512], f32, tag="ctxA")
            mix = psp.tile([128, 512], f32, tag="mix")


            # small DMAs ride the SWDGE ring
            nc.gpsimd.dma_start(bq_sb[:], bq[:])
            nc.gpsimd.dma_start(bk_sb[:], bk[:])
            nc.gpsimd.dma_start(bvr_sb[:], bvr[:])
            nc.gpsimd.memset(ones_row[:], 1.0)

            # weights first on the sync ring: tiny, needed before projections
            wk_t = wp.tile([128, 4, FSL], bf16, tag="wk_t")
            nc.sync.dma_start(wk_t[:], wkT.rearrange("(i p) f -> p i f", p=128))
            wq_t = wp.tile([128, 4, FSL], bf16, tag="wq_t")
            nc.sync.dma_start(wq_t[:], wqT.rearrange("(i p) f -> p i f", p=128))
            wv_t = wp.tile([128, 4, 130], bf16, tag="wv_t")
            nc.sync.dma_start(wv_t[:], wvT.rearrange("(i p) f -> p i f", p=128))
            nc.sync.dma_start(wo_sb[:], woT[:])

            vp_tiles = []

            def emit_qk_exp(qc, h, tbs):
                # one score tile covering up to 3 t-blocks; [128, 3, 512]
                # tiles (3 banks, bufs=2) amortize the exp instruction
                # overhead over 1536 lanes while keeping pipeline depth 2
                st = stp.tile([128, 2, QC], f32, tag="stA", bufs=3,
                              name="st")
                pt = ptp.tile([128, 2, QC], bf16, tag="ptA", bufs=66,
                              name="pt")
                n = len(tbs)
                for j, tb in enumerate(tbs):
                    nc.tensor.matmul(
                        st[:, j, :],
                        kt8[:, h, :, tb * 128:(tb + 1) * 128],
                        qt8[:, h, :, qc * QC:(qc + 1) * QC],
                        start=True, stop=True, perf_mode=PM.DoubleRow)
                pat = EXP_PATTERN_A if exp_phase[0] == "A" else EXP_PATTERN_B
                eng = pat[exp_idx[0] % len(pat)]
                exp_idx[0] += 1
                if eng == "A":
                    nc.scalar.activation(pt[:, 0:n, :], st[:, 0:n, :], AF.Exp,
                                         scale=float(HD) ** -0.5)
                else:
                    nc.vector.tensor_scalar(pt[:, 0:n, :].bitcast(i16),
                                            st[:, 0:n, :],
                                            EXP_S, EXP_B, ALU.mult, ALU.add)
                for j, tb in enumerate(tbs):
                    pt_map[(qc, h, tb)] = pt[:, j, :]

            def emit_pv_tail(qc, filler=()):
                # filler: (qc', h, tbs) QK/exp emissions interleaved between
                # PV groups to keep all engines fed
                filler = list(filler)
                nfill = len(filler)
                fi = 0
                ctxn = [sp.tile([128, 2, HD], bf16, tag=f"cn{sb}",
                                name=f"cn{sb}") for sb in range(4)]
                ctxTs = sp.tile([128, 512], bf16, tag="ctxTs", name="ctxTs")
                for g in range(8):
                    sb, h = g // 2, g % 2
                    slot = ctxA[:, 0:65]
                    for tb in range(NTB):
                        pt = pt_map[(qc, h, tb)]
                        nc.tensor.matmul(
                            slot,
                            pt[:, sb * 128:(sb + 1) * 128],
                            vp_tiles[tb][:, h * 65:(h + 1) * 65],
                            start=(tb == 0), stop=(tb == NTB - 1))
                    # stage the slot out fast so the next group can start;
                    # normalize off-bank from the SBUF copy
                    cg = sp.tile([128, 65], f32, tag="cg", bufs=3, name="cg")
                    nc.scalar.copy(cg[:], slot)
                    r1 = sp.tile([128, 1], f32, tag="r1", bufs=3, name="r1")
                    nc.vector.reciprocal(r1[:], cg[:, 64:65])
                    if g % 2 == 0:
                        nc.scalar.mul(ctxn[sb][:, h, :], cg[:, 0:HD], r1[:])
                    else:
                        nc.vector.tensor_scalar(
                            ctxn[sb][:, h, :], cg[:, 0:HD], r1[:],
                            None, ALU.mult)
                    if h == 1:
                        # [128s, (2h,64d)] -> [(2h,64d), 128s] on the DMA xbar
                        nc.sync.dma_start_transpose(
                            ctxTs[:, sb * 128:(sb + 1) * 128], ctxn[sb][:])
                        col = qc * QC + sb * 128
                        nc.tensor.matmul(
                            mix[:, :], ctxTs[:, sb * 128:(sb + 1) * 128],
                            wo_sb[:], start=True, stop=True)
                        ob = obp.tile([128, HID], bf16, tag="ob", name="ob")
                        if sb % 2 == 0:
                            nc.scalar.copy(ob[:], mix[:, :])
                        else:
                            nc.vector.tensor_copy(ob[:], mix[:, :])
                        nc.sync.dma_start(out[col:col + 128, :], ob[:])
                    want = nfill * (g + 1) // 8
                    while fi < want:
                        emit_qk_exp(*filler[fi])
                        fi += 1

            # ---------------- phase A: projections + qc0 QK/exp ------------
            qc0_cursor = [0]

            def qc0_chunks_upto(tb_max):
                outl = []
                while True:
                    tb = qc0_cursor[0]
                    n = min(2, NTB - tb, tb_max - tb)
                    if n <= 0 or (n < 2 and tb + n < NTB):
                        break
                    outl.append(tuple(range(tb, tb + n)))
                    qc0_cursor[0] += n
                return outl

            for t8 in range(NQC):
                cs = slice(t8 * QC, (t8 + 1) * QC)
                xc = wp.tile([128, 4, QC], bf16, tag="xc", bufs=3)
                nc.sync.dma_start(
                    xc[:], xT[:, cs].rearrange("(i p) t -> p i t", p=128))
                xcs = [xc[:, i, :] for i in range(4)]
                for w_t, bias_sb, dst, dst8, pbank in (
                        (wk_t, bk_sb, ktf, kt8, ctxA), (wq_t, bq_sb, qtf, qt8, mix)):
                    pb = pbank[:, :]
                    for i in range(4):
                        nc.tensor.matmul(
                            pb, w_t[:, i, :], xcs[i],
                            start=(i == 0), stop=(i == 3))
                    nc.scalar.add(dst[:, cs], pb, bias_sb[:])
                    for h in range(2):
                        for j in range(2):
                            eng = nc.gpsimd if j == 0 else nc.sync
                            eng.dma_start(
                                dst8[:, h, j, cs],
                                dst[h * 64 + j * 32:h * 64 + j * 32 + 32, cs])
                for tl in range(4):
                    tb = t8 * 4 + tl
                    vps = stp.tile([128, 2, QC], f32, tag="stA", bufs=3,
                                   name="vps")[:, 0, 0:130]
                    for i in range(4):
                        nc.tensor.matmul(
                            vps, xcs[i][:, tl * 128:(tl + 1) * 128], wv_t[:, i, :],
                            start=(i == 0), stop=False)
                    nc.tensor.matmul(vps, ones_row[:], bvr_sb[:],
                                     start=False, stop=True)
                    vp = vpp.tile([128, 130], bf16, tag="vp")
                    if tb % 2 == 0:
                        nc.scalar.copy(vp[:], vps)
                    else:
                        nc.vector.tensor_copy(vp[:], vps)
                    vp_tiles.append(vp)
                # qc0 scores/exp for every complete triple now available
                for tbs in qc0_chunks_upto(4 * t8 + 4):
                    for h in range(2):
                        emit_qk_exp(0, h, tbs)
            exp_phase[0] = "B"

            # ---------------- phase B: attention + output projection -------
            def chunked(tb0, off=0):
                outl, tb = [], tb0
                while tb < NTB:
                    n = min(2, NTB - tb)
                    outl.append(tuple(range(tb, tb + n)))
                    tb += n
                return outl

            for qc in range(NQC):
                filler = []
                if qc + 1 < NQC:
                    ts = 2 * Q1_EARLY if qc + 1 == 1 else 0
                    filler = [(qc + 1, h, tbs)
                              for tbs in chunked(ts) for h in range(2)]
                emit_pv_tail(qc, filler)

    nc.compile()
    return nc


def _get_program():
    global _PROGRAM
    if _PROGRAM is None:
        _PROGRAM = _build_program()
    return _PROGRAM


def kernel(**inputs):
    _ensure_imports()
    import ml_dtypes
    from concourse import bass_utils

    bf = ml_dtypes.bfloat16
    x = np.ascontiguousarray(np.asarray(inputs["x"], dtype=np.float32))
    Wq = np.asarray(inputs["Wq"], dtype=np.float32)
    Wk = np.asarray(inputs["Wk"], dtype=np.float32)
    Wv = np.asarray(inputs["Wv"], dtype=np.float32)
    Wo = np.asarray(inputs["Wo"], dtype=np.float32)
    bq = np.asarray(inputs["bq"], dtype=np.float32)
    bk = np.asarray(inputs["bk"], dtype=np.float32)
    bv = np.asarray(inputs["bv"], dtype=np.float32)
    bo = np.asarray(inputs["bo"], dtype=np.float32)

    nc = _get_program()

    wqT_full = np.ascontiguousarray(Wq.T)
    wkT_full = np.ascontiguousarray(Wk.T)
    wvT_full = np.ascontiguousarray(Wv.T)
    woT_full = np.ascontiguousarray(Wo.T)

    in_maps = []
    for c in range(NCORES):
        b, g = divmod(c, 4)
        fs = slice(g * FSL, (g + 1) * FSL)
        wv_aug = np.zeros((HID, 130), np.float32)
        wv_aug[:, 0:64] = wvT_full[:, g * FSL:g * FSL + 64]
        wv_aug[:, 65:129] = wvT_full[:, g * FSL + 64:(g + 1) * FSL]
        bv_aug = np.zeros((1, 130), np.float32)
        bv_aug[0, 0:64] = bv[g * FSL:g * FSL + 64]
        bv_aug[0, 64] = 1.0
        bv_aug[0, 65:129] = bv[g * FSL + 64:(g + 1) * FSL]
        bv_aug[0, 129] = 1.0
        in_maps.append({
            "xT": np.ascontiguousarray(x[b].T.astype(bf)),
            "wqT": np.ascontiguousarray(wqT_full[:, fs].astype(bf)),
            "wkT": np.ascontiguousarray(wkT_full[:, fs].astype(bf)),
            "wvT": np.ascontiguousarray(wv_aug.astype(bf)),
            "woT": np.ascontiguousarray(woT_full[fs, :].astype(bf)),
            "bq": np.ascontiguousarray(bq[fs].reshape(FSL, 1)),
            "bk": np.ascontiguousarray(bk[fs].reshape(FSL, 1)),
            "bvr": bv_aug.astype(bf),
        })

    res = bass_utils.run_bass_kernel_spmd(nc, in_maps,
                                          core_ids=list(range(NCORES)))
    outs = [np.asarray(r["out"], dtype=np.float32) for r in res.results]

    full = np.empty((B, S, HID), dtype=np.float32)
    for b in range(B):
        full[b] = outs[4 * b] + outs[4 * b + 1] + outs[4 * b + 2] + outs[4 * b + 3]
        full[b] += bo
    return full
